# revision 42
# baseline (speedup 1.0000x reference)
"""KQEnergyBlock Trainium2 Bass kernel, v3 (fp8 DoubleRow redesign).

Math per batch element (see reference):
  Q = x Wq^T, K = x Wk^T            (N, D), heads = 64-col slices
  S_h = beta_h Q_h K_h^T ; A_h = softmax(S_h)
  T1 = AVc @ Wq   (AVc  = concat_h A_h K_h)
  T2 = ATQc @ Wk  (ATQc = concat_h A_h^T Q_h)
  out = T1 + T2 + relu(x Wm^T) Wm

Cost-model facts driving the design (TimelineSim):
  - matmul engine time = out_free_size * cycles_per_row; fp8e4 (e4m3) with
    perf_mode=DoubleRow costs 0.5 cycles/row while contracting [K,2,*]
    operand pairs -> 4x bf16 MAC throughput when pairs carry real data.
  - ACT exp costs ~1038ns per [128,1024] instruction: the 12 M softmax
    exps (~100us) are the floor; everything else overlaps around it.
  - XBAR DMA transpose costs 14ns per 16x128 tile and requires a 2-byte
    dtype: transposing fp8 PAIRS bitcast as uint16 halves the tile count
    AND lands A^T pre-paired in exactly the [K,2,M] layout DoubleRow wants.

Numerics (threshold 2e-2; measured ~1.3e-2 in simulation):
  - mlp is ~96% of output norm, so mlp1 runs in bf16 (kills x- and W1-
    quantization error, and bf16 needs no DR pairs so it is cheaper than a
    compensated fp8 path).
  - stage4 hid@Wm runs fp8-DR with a host-prepared residual-weight pass
    (wm8c = fp8(SW*C*Wm - wm8b)) accumulated into the same psum.
  - weights scaled by SW=32 (keeps everything under fp8e4m3's 240 max),
    softmax renorm constant C=16 folded into Qs / AV-evict / final evict.

Sharding: data-parallel over batch B=8, one element per core, no
collectives.
"""

import numpy as np
import ml_dtypes

import concourse.mybir as mybir
import concourse.tile as tile
from concourse import bacc
from concourse.bass_utils import run_bass_kernel_spmd

B, N, D = 8, 1024, 768
H, Z = 12, 64
HID = 3072
P = 128
DC = D // P      # 6
NC = N // P      # 8
HC = HID // P    # 24
BF = mybir.dt.bfloat16
F32 = mybir.dt.float32
F8 = mybir.dt.float8e4
U16 = mybir.dt.uint16
Exp = mybir.ActivationFunctionType.Exp
DR = mybir.MatmulPerfMode.DoubleRow
Mult = mybir.AluOpType.mult

NPBF = ml_dtypes.bfloat16
NPF8 = ml_dtypes.float8_e4m3

SW = 32.0        # weight scale into fp8
C = 16.0         # softmax renorm constant

_CACHE = {}


def _build(dbg=False):
    nc = bacc.Bacc("TRN2", target_bir_lowering=False, debug=False, num_devices=8)
    dbg_d = {}
    if dbg:
        for nm, shp, dt in (("QT8", [P, DC, 2, N], F8), ("KT8", [P, DC * N + 256], F8),
                            ("E80", [P, NC, N], F8), ("ET80", [P, 32, P], U16),
                            ("Qn8", [P, NC, D], F8), ("Knp", [P, DC, 4, P], U16),
                            ("hid8", [P, HC, N], F8), ("AVT8", [P, DC, NC, P], F8),
                            ("ATQT8", [P, DC, NC, P], F8), ("r0", [P, NC], F32)):
            dbg_d[nm] = nc.dram_tensor("dbg_" + nm, shp, dt, kind="ExternalOutput")

    xT8c_d = nc.dram_tensor("xT8c", [D, 2, N], F8, kind="ExternalInput")
    wqT8_d = nc.dram_tensor("wqT8", [D, D], F8, kind="ExternalInput")
    wkT8_d = nc.dram_tensor("wkT8", [D, D], F8, kind="ExternalInput")
    wq8_d = nc.dram_tensor("wq8", [D, D], F8, kind="ExternalInput")
    wk8_d = nc.dram_tensor("wk8", [D, D], F8, kind="ExternalInput")
    wmT8_d = nc.dram_tensor("wmT8", [D, HID], F8, kind="ExternalInput")
    wmT8c_d = nc.dram_tensor("wmT8c", [D, 2, HID], F8, kind="ExternalInput")
    wm8b_d = nc.dram_tensor("wm8b", [HID, D], F8, kind="ExternalInput")
    wm8c_d = nc.dram_tensor("wm8c", [HID, D], F8, kind="ExternalInput")
    betap_d = nc.dram_tensor("betap", [P, H], F32, kind="ExternalInput")
    ident_d = nc.dram_tensor("ident", [P, P], BF, kind="ExternalInput")
    ident8_d = nc.dram_tensor("ident8", [P, P], F8, kind="ExternalInput")
    out_d = nc.dram_tensor("out", [N, D], BF, kind="ExternalOutput")

    xT8c_v = xT8c_d.ap().rearrange("(c p) a n -> p c a n", p=P)  # [128,6,2,1024]
    wqT8_v = wqT8_d.ap().rearrange("(c p) e -> p c e", p=P)
    wkT8_v = wkT8_d.ap().rearrange("(c p) e -> p c e", p=P)
    wq8_v = wq8_d.ap().rearrange("(c p) d -> p c d", p=P)
    wk8_v = wk8_d.ap().rearrange("(c p) d -> p c d", p=P)
    wmT8_v = wmT8_d.ap().rearrange("(c p) h -> p c h", p=P)   # [128, 6, 3072]
    wmT8c_v = wmT8c_d.ap().rearrange("(c p) a h -> p c a h", p=P)
    wm8b_v = wm8b_d.ap().rearrange("(c p) d -> p c d", p=P)   # [128, 24, 768]
    wm8c_v = wm8c_d.ap().rearrange("(c p) d -> p c d", p=P)
    out_v = out_d.ap().rearrange("(c p) d -> p c d", p=P)     # [128, 8, 768]

    with tile.TileContext(nc) as tc:
        with (
            tc.tile_pool(name="acts", bufs=1) as acts,
            tc.tile_pool(name="hd", bufs=1) as hd,
            tc.tile_pool(name="stream", bufs=1) as stream,
            tc.tile_pool(name="ps", bufs=1, space="PSUM") as ps,
        ):
            # ---- persistent tiles ----
            xT8c = acts.tile([P, DC, 2, N], F8)
            wqT8 = acts.tile([P, DC, D], F8)
            wkT8 = acts.tile([P, DC, D], F8)
            wq8 = acts.tile([P, DC, D], F8)
            wk8 = acts.tile([P, DC, D], F8)
            wm8b = acts.tile([P, HC, D], F8)
            wm8c = acts.tile([P, HC, D], F8)
            betap = acts.tile([P, H], F32)
            ident = acts.tile([P, P], BF)
            ident8 = acts.tile([P, P], F8)
            ones8 = acts.tile([P, 2, 16], F8)
            # projections: QT8 slot1 is a zero pad (DoubleRow partner row);
            # KT8 gets a 256-col zero tail so the k-window rhs can overrun.
            QT8 = acts.tile([P, DC, 2, N], F8)
            KT8 = acts.tile([P, DC * N + 256], F8)
            Qn8 = acts.tile([P, NC, D], F8)
            Knp = acts.tile([P, DC, 4, P], U16)     # paired Kn from XBAR
            hid8 = acts.tile([P, HC, N], F8)
            # Qs: persistent [q, qo, 128]; head h writes z-cols
            # [(h%2)*64, +64) so consecutive heads use disjoint halves and
            # the ATQ stationary can span all 128 columns (dual-fp8 matmuls
            # must write psum starting at partition 0).
            Qs8 = acts.tile([P, NC, P], F8)
            AVT8 = acts.tile([P, DC, NC, P], F8)
            ATQT8 = acts.tile([P, DC, NC, P], F8)

            def psE():
                return ps.tile([P, N], F32, tag="psE", name="pt", bufs=2)

            def ps_pav():
                return ps.tile([P, NC, Z], F32, tag="pav", name="pav", bufs=2)

            def ps_atq():
                # [:, 0:256] = ATQT kseg accumulator; [:, 256:264] doubles as
                # the rowsum accumulator (separate allocations, same ring).
                return ps.tile([P, 264], F32, tag="patq", name="patq", bufs=2)

            # ---- input loads (ordered for proj critical path) ----
            # wait-free input loads ride the ACT HWDGE queue: they dispatch
            # immediately (nothing to wait on) and keep the SP queue clear
            # for the data-dependent XBAR transposes of the early heads.
            nc.scalar.dma_start(wqT8[:], wqT8_v)
            nc.scalar.dma_start(xT8c[:, :, 0, :], xT8c_v[:, :, 0, :])
            nc.scalar.dma_start(wkT8[:], wkT8_v)
            nc.scalar.dma_start(xT8c[:, :, 1, :], xT8c_v[:, :, 1, :])
            nc.scalar.dma_start(betap[:], betap_d.ap())
            nc.scalar.dma_start(ident[:], ident_d.ap())
            nc.scalar.dma_start(ident8[:], ident8_d.ap())
            nc.gpsimd.memset(QT8[:, :, 1, :], 0.0)
            nc.gpsimd.memset(KT8[:, DC * N:], 0.0)
            nc.vector.memset(ones8[:], 1.0)
            nc.gpsimd.memset(Qs8[:, :, Z:], 0.0)

            KT8v = KT8[:]  # [128, 6400]

            # ---- projections: QT/KT feature-major, fp8 DoubleRow ----
            for eo in range(DC):
                for wT, isq in ((wqT8, True), (wkT8, False)):
                    pt = psE()
                    # segment-major: a region's full start->stop accumulation
                    # chain must complete before the next start in the same
                    # 2KB psum zero-region
                    for ns in range(4):
                        for cp in range(DC // 2):
                            nc.tensor.matmul(
                                pt[:, ns * 256:(ns + 1) * 256],
                                wT[:, 2 * cp:2 * cp + 2, eo * P:(eo + 1) * P],
                                xT8c[:, 2 * cp:2 * cp + 2, 0,
                                     ns * 256:(ns + 1) * 256],
                                start=(cp == 0), stop=(cp == DC // 2 - 1),
                                perf_mode=DR,
                            )
                    if isq:
                        nc.vector.tensor_copy(QT8[:, eo, 0, :], pt[:])
                    else:
                        nc.vector.tensor_copy(
                            KT8v[:, eo * N:(eo + 1) * N], pt[:])

            # Qn: natural-layout Q via PE transposes (fp8). FP8 transpose
            # writes with element step 2; evict reads the even-byte lanes.
            for eo in range(DC):
                ptb = psE()[:].bitcast(F8)   # [128, 4096]
                ptv = ptb[:, 0:NC * 2 * P].rearrange(
                    "p (a f s) -> p a s f", a=NC, s=2)   # [128, 8, 2, 128]
                for qo in range(NC):
                    nc.tensor.transpose(
                        ptv[:, qo, 0, :],
                        QT8[:, eo, 0, qo * P:(qo + 1) * P], ident8[:])
                nc.vector.tensor_copy(
                    Qn8[:, :, eo * P:(eo + 1) * P], ptv[:, :, 0, :])

            # ---- mlp1 (fp8 DR): hid8[ho] = relu(Wm x^T) in four 256-wide
            # chains per ho on the patq psum ring. Each chain: 3 main
            # DoubleRow instrs (c-pairs) + 6 correction instrs whose DR
            # slots carry (dW1[c], W1/16[c]) x (x8[c], 16*dx[c]) — removing
            # both the weight- and activation-quantization error of the
            # main pass at half of bf16's cycle cost.
            def mlp1_emit(ho):
                if ho % 4 == 0:
                    w4 = stream.tile([P, DC, 4 * P], F8, tag="wmT", name="wt",
                                     bufs=2)
                    nc.scalar.dma_start(
                        w4[:], wmT8_v[:, :, ho * P:(ho + 4) * P])
                    wc4 = stream.tile([P, DC, 2, 4 * P], F8, tag="wmTc",
                                      name="wtc", bufs=2)
                    for a in range(2):
                        nc.scalar.dma_start(
                            wc4[:, :, a, :],
                            wmT8c_v[:, :, a, ho * P:(ho + 4) * P])
                    mlp1_emit.w4, mlp1_emit.wc4 = w4, wc4
                o4 = (ho % 4) * P
                wt = mlp1_emit.w4[:, :, o4:o4 + P]
                wtc = mlp1_emit.wc4[:, :, :, o4:o4 + P]

                def chain(ns, ho=ho, wt=wt, wtc=wtc):
                    pt = ps_atq()[:, 0:256]
                    nsl = slice(ns * 256, (ns + 1) * 256)
                    for cp in range(DC // 2):
                        nc.tensor.matmul(
                            pt[:], wt[:, 2 * cp:2 * cp + 2, :],
                            xT8c[:, 2 * cp:2 * cp + 2, 0, nsl],
                            start=(cp == 0), stop=False, perf_mode=DR,
                        )
                    for c in range(DC):
                        nc.tensor.matmul(
                            pt[:], wtc[:, c, :, :], xT8c[:, c, :, nsl],
                            start=False, stop=(c == DC - 1), perf_mode=DR,
                        )
                    nc.vector.tensor_scalar_max(
                        hid8[:, ho, nsl], pt[:], 0.0)
                return [lambda ns=ns: chain(ns) for ns in range(4)]

            # ---- per-head attention pieces ----
            def s_exp(h, filler):
                """E8 = exp(beta/SW^2 * S) in fp8, S via zero-padded DR."""
                zo = (h % 2) * Z
                c = h // 2
                E8 = hd.tile([P, NC, N], F8, tag="E8", name="E8", bufs=2)
                for qo in range(NC):
                    pt = psE()
                    for j in range(4):
                        nc.tensor.matmul(
                            pt[:, j * 256:(j + 1) * 256],
                            QT8[zo:zo + Z, c, :, qo * P:(qo + 1) * P],
                            KT8v[zo:zo + Z, c * N + j * 256:c * N + j * 256 + 512]
                            .rearrange("p (a b) -> p a b", a=2),
                            start=True, stop=True, perf_mode=DR,
                        )
                    nc.scalar.activation(
                        E8[:, qo, :], pt[:], Exp, scale=betap[:, h:h + 1])
                    f = next(filler, None)
                    if f is not None:
                        f()
                return E8

            def et_xbar(h, E8):
                ET8 = hd.tile([P, 32, P], U16, tag="ET8", name="ET8", bufs=2)
                nc.sync.dma_start(
                    ET8[:], E8[:].bitcast(U16).rearrange("p a b -> p (a b)"),
                    transpose=True)
                return ET8

            def et_lhs(ET8, qo, kcp, s):
                # A^T stationary: [128 kp, 2 (kc-pair, 256B stride), 128 q
                # (step 2, parity s)] — dual-fp8 LdWeights needs the pair
                # stride 16B-aligned, so the byte-interleave is consumed as
                # two parity lanes instead.
                sl = ET8[:].bitcast(F8)[:, qo * 4 + 2 * kcp:qo * 4 + 2 * kcp + 2, :]
                return sl.rearrange("p a (f s) -> p a s f", s=2)[:, :, s, :]

            def rowsum(h, ET8):
                """r[q] per qo via tiny PE ones-matmuls on packed A^T."""
                pr = ps_atq()[:, 256:256 + NC]
                for qo in range(NC):
                    for i, (kcp, s) in enumerate(
                            ((0, 0), (0, 1), (1, 0), (1, 1))):
                        nc.tensor.matmul(
                            pr[:, qo:qo + 1], et_lhs(ET8, qo, kcp, s),
                            ones8[:, :, 0:1],
                            start=(i == 0), stop=(i == 3), perf_mode=DR,
                        )
                return pr

            def make_qs(h, pr):
                """rc = C/r ; Qs[z-half of head h] = Qn * rc (fp8)."""
                rc = hd.tile([P, NC], F32, tag="rc", name="rc", bufs=2)
                zo = (h % 2) * Z
                nc.vector.reciprocal(rc[:], pr[:])
                for qo in range(NC):
                    nc.vector.tensor_scalar(
                        Qs8[:, qo, zo:zo + Z], Qn8[:, qo, h * Z:(h + 1) * Z],
                        rc[:, qo:qo + 1], C, Mult, Mult)
                return rc

            def atq_direct(h, E8):
                """ATQT[z,k] = sum_q Qs[q,z] E8[q,k], evicted per kseg.
                The stationary spans all 128 Qs columns; the other head's
                64 rows compute junk that the evict skips."""
                zo = (h % 2) * Z
                c = h // 2
                for ks in range(4):
                    pt = ps_atq()[:, 0:256]
                    for qp in range(4):
                        nc.tensor.matmul(
                            pt[:],
                            Qs8[:, 2 * qp:2 * qp + 2, :],
                            E8[:, 2 * qp:2 * qp + 2, ks * 256:(ks + 1) * 256],
                            start=(qp == 0), stop=(qp == 3), perf_mode=DR,
                        )
                    nc.vector.tensor_copy(
                        ATQT8[zo:zo + Z, c, 2 * ks:2 * ks + 2, :],
                        pt[zo:zo + Z, :].rearrange("p (a b) -> p a b", b=P))

            def av_part(h, ET8):
                """AV[q,z] = sum_k A^T[k,q] K[k,z] via packed ET8 + Knp."""
                c = h // 2
                if h % 2 == 0:
                    av_part.pav_e = ps_pav()
                    pav = av_part.pav_e
                else:
                    av_part.pav_o = ps_pav()
                    pav = av_part.pav_o
                Knpf = Knp[:].bitcast(F8)   # [128, 6, 4, 256]
                for qo in range(NC):
                    for i, (kcp, s) in enumerate(
                            ((0, 0), (0, 1), (1, 0), (1, 1))):
                        rhs = Knpf[:, c, 2 * kcp:2 * kcp + 2, :].rearrange(
                            "p a (f s) -> p a s f", s=2)[
                                :, :, s, (h % 2) * Z:(h % 2) * Z + Z]
                        nc.tensor.matmul(
                            pav[:, qo, :], et_lhs(ET8, qo, kcp, s), rhs,
                            start=(i == 0), stop=(i == 3), perf_mode=DR,
                        )

            def av_evict(hpair, rc_e, rc_o):
                """An = pav * (C/r) per head, PE-transpose into AVT8."""
                c = hpair
                An8 = hd.tile([P, NC, 2 * Z], F8, tag="An8", name="An8", bufs=2)
                for qo in range(NC):
                    nc.vector.tensor_scalar(
                        An8[:, qo, 0:Z], av_part.pav_e[:, qo, :],
                        rc_e[:, qo:qo + 1], C, Mult, Mult)
                    nc.vector.tensor_scalar(
                        An8[:, qo, Z:2 * Z], av_part.pav_o[:, qo, :],
                        rc_o[:, qo:qo + 1], C, Mult, Mult)
                ptb = psE()[:].bitcast(F8)
                ptv = ptb[:, 0:NC * 2 * P].rearrange(
                    "p (a f s) -> p a s f", a=NC, s=2)   # [128, 8, 2, 128]
                for qo in range(NC):
                    nc.tensor.transpose(
                        ptv[:, qo, 0, :], An8[:, qo, :], ident8[:])
                nc.vector.tensor_copy(AVT8[:, c, :, :], ptv[:, :, 0, :])

            # ---- software pipeline over heads ----
            # phase h: S(h)+exp(h) with mlp1 fillers; then rowsum/Qs/ATQ/AV
            # for h-1 (its XBAR landed during exp(h)); pair-evict at odd h-1.
            mlp_chunks = [mlp1_emit(ho) for ho in range(2)]
            # Knp XBARs wait on the KT8 evicts; emitted after the first mlp
            # block loads so those aren't head-of-line blocked on SP.
            KT8u = KT8[:].bitcast(U16)       # [128, 3200]
            for c in range(DC):
                nc.sync.dma_start(
                    Knp[:, c, :, :],
                    KT8u[:, c * 512:(c + 1) * 512], transpose=True)
            prev = None           # (E8, ET8) of h-1
            rcs = {}
            next_ho = 2
            for h in range(H + 1):
                filler_items = []
                if h < H:
                    # two mlp1 chunks (8 chains) per phase: one chain per qo
                    for _ in range(2):
                        if mlp_chunks:
                            filler_items.extend(mlp_chunks.pop(0))
                        if next_ho < HC:
                            mlp_chunks.append(mlp1_emit(next_ho))
                            next_ho += 1
                    filler = iter(filler_items)
                    E8 = s_exp(h, filler)
                    for f in filler:
                        f()
                if prev is not None:
                    hp = h - 1
                    pE8, pET8 = prev
                    pr = rowsum(hp, pET8)
                    rcs[hp] = make_qs(hp, pr)
                    atq_direct(hp, pE8)
                    av_part(hp, pET8)
                    if hp % 2 == 1:
                        av_evict(hp // 2, rcs[hp - 1], rcs[hp])
                        if dbg and hp == 1:
                            nc.sync.dma_start(dbg_d["r0"].ap(), rcs[0][:])
                if h < H:
                    ET8 = et_xbar(h, E8)
                    prev = (E8, ET8)
                    if dbg and h == 0:
                        nc.sync.dma_start(dbg_d["E80"].ap(), E8[:])
                        nc.sync.dma_start(dbg_d["ET80"].ap(), ET8[:])
                if h == 7:
                    nc.scalar.dma_start(wq8[:], wq8_v)
                if h == 8:
                    nc.scalar.dma_start(wk8[:], wk8_v)
                if h == 9:
                    nc.scalar.dma_start(wm8b[:], wm8b_v)
                if h == 10:
                    nc.scalar.dma_start(wm8c[:], wm8c_v)

            if dbg:
                nc.sync.dma_start(dbg_d["QT8"].ap(), QT8[:])
                nc.sync.dma_start(dbg_d["KT8"].ap(), KT8[:])
                nc.sync.dma_start(dbg_d["Qn8"].ap(), Qn8[:])
                nc.sync.dma_start(dbg_d["Knp"].ap(), Knp[:])
                nc.sync.dma_start(dbg_d["hid8"].ap(), hid8[:])
                nc.sync.dma_start(dbg_d["AVT8"].ap(), AVT8[:])
                nc.sync.dma_start(dbg_d["ATQT8"].ap(), ATQT8[:])

            # ---- stage 4: out = (AVc@Wq + ATQc@Wk + hid@(Wm*C)) / (SW^2 C)
            OSC = 1.0 / (SW * SW * C)
            for rnd in range(4):
                pouts = [psE() for _ in range(2)]
                for i, po in enumerate(pouts):
                    no = 2 * rnd + i
                    pt = po[:, 0:D]
                    for ds in range(3):
                        dsl = slice(ds * 256, (ds + 1) * 256)
                        for cp in range(DC // 2):
                            for lhsT, w in ((AVT8, wq8), (ATQT8, wk8)):
                                nc.tensor.matmul(
                                    pt[:, dsl],
                                    lhsT[:, 2 * cp:2 * cp + 2, no, :],
                                    w[:, 2 * cp:2 * cp + 2, dsl],
                                    start=(cp == 0 and lhsT is AVT8),
                                    stop=False, perf_mode=DR,
                                )
                        for wm_t in (wm8b, wm8c):
                            for hp in range(HC // 2):
                                nc.tensor.matmul(
                                    pt[:, dsl],
                                    hid8[:, 2 * hp:2 * hp + 2, no * P:(no + 1) * P],
                                    wm_t[:, 2 * hp:2 * hp + 2, dsl],
                                    start=False,
                                    stop=(wm_t is wm8c and hp == HC // 2 - 1),
                                    perf_mode=DR,
                                )
                osb = stream.tile([P, 2, D], BF, tag="osb", name="osb", bufs=2)
                for i in range(2):
                    nc.scalar.activation(
                        osb[:, i, :], pouts[i][:, 0:D],
                        mybir.ActivationFunctionType.Copy, scale=OSC)
                nc.sync.dma_start(out_v[:, 2 * rnd:2 * rnd + 2, :], osb[:])

    nc.compile()
    return nc


def _prep(x, Wq, Wk, betas, W_mlp):
    x = np.asarray(x, dtype=np.float32)
    Wq = np.asarray(Wq, dtype=np.float32)
    Wk = np.asarray(Wk, dtype=np.float32)
    betas = np.asarray(betas, dtype=np.float32)
    W_mlp = np.asarray(W_mlp, dtype=np.float32)

    wqT8 = np.ascontiguousarray(Wq.T * SW).astype(NPF8)
    wkT8 = np.ascontiguousarray(Wk.T * SW).astype(NPF8)
    wq8 = np.ascontiguousarray(Wq * SW).astype(NPF8)
    wk8 = np.ascontiguousarray(Wk * SW).astype(NPF8)
    wmT8 = np.ascontiguousarray(W_mlp.T * SW).astype(NPF8)
    wmT8c = np.empty((D, 2, HID), NPF8)
    wmT8c[:, 0, :] = (W_mlp.T * SW - wmT8.astype(np.float32)).astype(NPF8)
    wmT8c[:, 1, :] = (wmT8.astype(np.float32) / 16.0).astype(NPF8)
    wm8b = np.ascontiguousarray(W_mlp * (SW * C)).astype(NPF8)
    wm8c = np.ascontiguousarray(
        W_mlp * (SW * C) - wm8b.astype(np.float32)).astype(NPF8)
    betap = np.ascontiguousarray(np.broadcast_to(
        (betas / (SW * SW))[None, :], (P, H))).astype(np.float32)
    ident = np.eye(P, dtype=np.float32).astype(NPBF)
    ident8 = np.eye(P, dtype=np.float32).astype(NPF8)

    in_maps = []
    for b in range(B):
        xT = np.ascontiguousarray(x[b].T)
        x8 = xT.astype(NPF8)
        xc = np.empty((D, 2, N), NPF8)
        xc[:, 0, :] = x8
        xc[:, 1, :] = (16.0 * (xT - x8.astype(np.float32))).astype(NPF8)
        in_maps.append({
            "xT8c": xc,
            "wqT8": wqT8, "wkT8": wkT8, "wq8": wq8, "wk8": wk8,
            "wmT8": wmT8, "wmT8c": wmT8c, "wm8b": wm8b, "wm8c": wm8c,
            "betap": betap, "ident": ident, "ident8": ident8,
        })
    return in_maps


def kernel(x, Wq, Wk, betas, W_mlp, _trace=False, _dbg=False):
    key = "nc_dbg" if _dbg else "nc"
    if key not in _CACHE:
        _CACHE[key] = _build(dbg=_dbg)
    nc = _CACHE[key]
    in_maps = _prep(x, Wq, Wk, betas, W_mlp)
    core_ids = list(range(B)) if not _dbg else [0]
    res = run_bass_kernel_spmd(nc, in_maps[:len(core_ids)], core_ids=core_ids,
                               trace=_trace)
    _CACHE["last_result"] = res
    if _dbg:
        return res
    out = np.stack([res.results[b]["out"] for b in range(B)], axis=0)
    return out.astype(np.float32)


# revision 43
# speedup vs baseline: 1.1836x; 1.1836x over previous
"""KQEnergyBlock Trainium2 Bass kernel, v3 (fp8 DoubleRow redesign).

Math per batch element (see reference):
  Q = x Wq^T, K = x Wk^T            (N, D), heads = 64-col slices
  S_h = beta_h Q_h K_h^T ; A_h = softmax(S_h)
  T1 = AVc @ Wq   (AVc  = concat_h A_h K_h)
  T2 = ATQc @ Wk  (ATQc = concat_h A_h^T Q_h)
  out = T1 + T2 + relu(x Wm^T) Wm

Cost-model facts driving the design (TimelineSim):
  - matmul engine time = out_free_size * cycles_per_row; fp8e4 (e4m3) with
    perf_mode=DoubleRow costs 0.5 cycles/row while contracting [K,2,*]
    operand pairs -> 4x bf16 MAC throughput when pairs carry real data.
  - ACT exp costs ~1038ns per [128,1024] instruction: the 12 M softmax
    exps (~100us) are the floor; everything else overlaps around it.
  - XBAR DMA transpose costs 14ns per 16x128 tile and requires a 2-byte
    dtype: transposing fp8 PAIRS bitcast as uint16 halves the tile count
    AND lands A^T pre-paired in exactly the [K,2,M] layout DoubleRow wants.

Numerics (threshold 2e-2; measured ~1.3e-2 in simulation):
  - mlp is ~96% of output norm, so mlp1 runs in bf16 (kills x- and W1-
    quantization error, and bf16 needs no DR pairs so it is cheaper than a
    compensated fp8 path).
  - stage4 hid@Wm runs fp8-DR with a host-prepared residual-weight pass
    (wm8c = fp8(SW*C*Wm - wm8b)) accumulated into the same psum.
  - weights scaled by SW=32 (keeps everything under fp8e4m3's 240 max),
    softmax renorm constant C=16 folded into Qs / AV-evict / final evict.

Sharding: data-parallel over batch B=8, one element per core, no
collectives.
"""

import numpy as np
import ml_dtypes

import concourse.mybir as mybir
import concourse.tile as tile
from concourse import bacc
from concourse.bass_utils import run_bass_kernel_spmd

B, N, D = 8, 1024, 768
H, Z = 12, 64
HID = 3072
P = 128
DC = D // P      # 6
NC = N // P      # 8
HC = HID // P    # 24
BF = mybir.dt.bfloat16
F32 = mybir.dt.float32
F8 = mybir.dt.float8e4
U16 = mybir.dt.uint16
Exp = mybir.ActivationFunctionType.Exp
DR = mybir.MatmulPerfMode.DoubleRow
Mult = mybir.AluOpType.mult

NPBF = ml_dtypes.bfloat16
NPF8 = ml_dtypes.float8_e4m3

SW = 32.0        # weight scale into fp8
C = 16.0         # softmax renorm constant

_CACHE = {}


def _build(dbg=False):
    nc = bacc.Bacc("TRN2", target_bir_lowering=False, debug=False, num_devices=8)
    dbg_d = {}
    if dbg:
        for nm, shp, dt in (("QT8", [P, DC, 2, N], F8), ("KT8", [P, DC * N + 256], F8),
                            ("E80", [P, NC, N], F8), ("ET80", [P, 32, P], U16),
                            ("Qn8", [P, NC, D], F8), ("Knp", [P, DC, 4, P], U16),
                            ("hid8", [P, HC, N], F8), ("AVT8", [P, DC, NC, P], F8),
                            ("ATQT8", [P, DC, NC, P], F8), ("r0", [P, NC], F32)):
            dbg_d[nm] = nc.dram_tensor("dbg_" + nm, shp, dt, kind="ExternalOutput")

    xT8c_d = nc.dram_tensor("xT8c", [D, 2, N], F8, kind="ExternalInput")
    wqT8_d = nc.dram_tensor("wqT8", [D, D], F8, kind="ExternalInput")
    wkT8_d = nc.dram_tensor("wkT8", [D, D], F8, kind="ExternalInput")
    wq8_d = nc.dram_tensor("wq8", [D, D], F8, kind="ExternalInput")
    wk8_d = nc.dram_tensor("wk8", [D, D], F8, kind="ExternalInput")
    wmT8_d = nc.dram_tensor("wmT8", [D, HID], F8, kind="ExternalInput")
    wmT8c_d = nc.dram_tensor("wmT8c", [D, 2, HID], F8, kind="ExternalInput")
    wm8b_d = nc.dram_tensor("wm8b", [HID, D], F8, kind="ExternalInput")
    wm8c_d = nc.dram_tensor("wm8c", [HID, D], F8, kind="ExternalInput")
    betap_d = nc.dram_tensor("betap", [P, H], F32, kind="ExternalInput")
    ident_d = nc.dram_tensor("ident", [P, P], BF, kind="ExternalInput")
    ident8_d = nc.dram_tensor("ident8", [P, P], F8, kind="ExternalInput")
    out_d = nc.dram_tensor("out", [N, D], BF, kind="ExternalOutput")

    xT8c_v = xT8c_d.ap().rearrange("(c p) a n -> p c a n", p=P)  # [128,6,2,1024]
    wqT8_v = wqT8_d.ap().rearrange("(c p) e -> p c e", p=P)
    wkT8_v = wkT8_d.ap().rearrange("(c p) e -> p c e", p=P)
    wq8_v = wq8_d.ap().rearrange("(c p) d -> p c d", p=P)
    wk8_v = wk8_d.ap().rearrange("(c p) d -> p c d", p=P)
    wmT8_v = wmT8_d.ap().rearrange("(c p) h -> p c h", p=P)   # [128, 6, 3072]
    wmT8c_v = wmT8c_d.ap().rearrange("(c p) a h -> p c a h", p=P)
    wm8b_v = wm8b_d.ap().rearrange("(c p) d -> p c d", p=P)   # [128, 24, 768]
    wm8c_v = wm8c_d.ap().rearrange("(c p) d -> p c d", p=P)
    out_v = out_d.ap().rearrange("(c p) d -> p c d", p=P)     # [128, 8, 768]

    with tile.TileContext(nc) as tc:
        with (
            tc.tile_pool(name="acts", bufs=1) as acts,
            tc.tile_pool(name="hd", bufs=1) as hd,
            tc.tile_pool(name="stream", bufs=1) as stream,
            tc.tile_pool(name="ps", bufs=1, space="PSUM") as ps,
        ):
            # ---- persistent tiles ----
            xT8c = acts.tile([P, DC, 2, N], F8)
            wqT8 = acts.tile([P, DC, D], F8)
            wkT8 = acts.tile([P, DC, D], F8)
            wq8 = acts.tile([P, DC, D], F8)
            wk8 = acts.tile([P, DC, D], F8)
            wm8b = acts.tile([P, HC, D], F8)
            wm8c = acts.tile([P, HC, D], F8)
            betap = acts.tile([P, H], F32)
            ident = acts.tile([P, P], BF)
            ident8 = acts.tile([P, P], F8)
            ones8 = acts.tile([P, 2, 16], F8)
            # projections: QT8 slot1 is a zero pad (DoubleRow partner row);
            # KT8 gets a 256-col zero tail so the k-window rhs can overrun.
            QT8 = acts.tile([P, DC, 2, N], F8)
            KT8 = acts.tile([P, DC * N + 256], F8)
            Qn8 = acts.tile([P, NC, D], F8)
            Knp = acts.tile([P, DC, 4, P], U16)     # paired Kn from XBAR
            hid8 = acts.tile([P, HC, N], F8)
            # Qs: persistent [q, qo, 128]; head h writes z-cols
            # [(h%2)*64, +64) so consecutive heads use disjoint halves and
            # the ATQ stationary can span all 128 columns (dual-fp8 matmuls
            # must write psum starting at partition 0).
            Qs8 = acts.tile([P, NC, P], F8)
            AVT8 = acts.tile([P, DC, NC, P], F8)
            ATQT8 = acts.tile([P, DC, NC, P], F8)

            def psE():
                return ps.tile([P, N], F32, tag="psE", name="pt", bufs=2)

            def ps_pav():
                return ps.tile([P, NC, Z], F32, tag="pav", name="pav", bufs=2)

            def ps_atq():
                # [:, 0:256] = ATQT kseg accumulator; [:, 256:264] doubles as
                # the rowsum accumulator (separate allocations, same ring).
                return ps.tile([P, 264], F32, tag="patq", name="patq", bufs=2)

            # ---- input loads (ordered for proj critical path) ----
            # wait-free input loads ride the ACT HWDGE queue: they dispatch
            # immediately (nothing to wait on) and keep the SP queue clear
            # for the data-dependent XBAR transposes of the early heads.
            nc.scalar.dma_start(wqT8[:], wqT8_v)
            nc.scalar.dma_start(xT8c[:, :, 0, :], xT8c_v[:, :, 0, :])
            nc.scalar.dma_start(wkT8[:], wkT8_v)
            nc.scalar.dma_start(xT8c[:, :, 1, :], xT8c_v[:, :, 1, :])
            nc.scalar.dma_start(betap[:], betap_d.ap())
            nc.scalar.dma_start(ident[:], ident_d.ap())
            nc.scalar.dma_start(ident8[:], ident8_d.ap())
            nc.gpsimd.memset(QT8[:, :, 1, :], 0.0)
            nc.gpsimd.memset(KT8[:, DC * N:], 0.0)
            nc.vector.memset(ones8[:], 1.0)
            nc.gpsimd.memset(Qs8[:, :, Z:], 0.0)

            KT8v = KT8[:]  # [128, 6400]

            # ---- projections: QT/KT feature-major, fp8 DoubleRow ----
            for eo in range(DC):
                for wT, isq in ((wqT8, True), (wkT8, False)):
                    pt = psE()
                    # segment-major: a region's full start->stop accumulation
                    # chain must complete before the next start in the same
                    # 2KB psum zero-region
                    for ns in range(4):
                        for cp in range(DC // 2):
                            nc.tensor.matmul(
                                pt[:, ns * 256:(ns + 1) * 256],
                                wT[:, 2 * cp:2 * cp + 2, eo * P:(eo + 1) * P],
                                xT8c[:, 2 * cp:2 * cp + 2, 0,
                                     ns * 256:(ns + 1) * 256],
                                start=(cp == 0), stop=(cp == DC // 2 - 1),
                                perf_mode=DR,
                            )
                    if isq:
                        nc.vector.tensor_copy(QT8[:, eo, 0, :], pt[:])
                    else:
                        nc.vector.tensor_copy(
                            KT8v[:, eo * N:(eo + 1) * N], pt[:])

            # Qn: natural-layout Q via PE transposes (fp8). FP8 transpose
            # writes with element step 2; evict reads the even-byte lanes.
            for eo in range(DC):
                ptb = psE()[:].bitcast(F8)   # [128, 4096]
                ptv = ptb[:, 0:NC * 2 * P].rearrange(
                    "p (a f s) -> p a s f", a=NC, s=2)   # [128, 8, 2, 128]
                for qo in range(NC):
                    nc.tensor.transpose(
                        ptv[:, qo, 0, :],
                        QT8[:, eo, 0, qo * P:(qo + 1) * P], ident8[:])
                nc.vector.tensor_copy(
                    Qn8[:, :, eo * P:(eo + 1) * P], ptv[:, :, 0, :])

            # ---- mlp1 (fp8 DR): hid8[ho] = relu(Wm x^T) in four 256-wide
            # chains per ho on the patq psum ring. Each chain: 3 main
            # DoubleRow instrs (c-pairs) + 6 correction instrs whose DR
            # slots carry (dW1[c], W1/16[c]) x (x8[c], 16*dx[c]) — removing
            # both the weight- and activation-quantization error of the
            # main pass at half of bf16's cycle cost.
            def mlp1_emit(ho):
                if ho % 4 == 0:
                    w4 = stream.tile([P, DC, 4 * P], F8, tag="wmT", name="wt",
                                     bufs=2)
                    nc.sync.dma_start(
                        w4[:], wmT8_v[:, :, ho * P:(ho + 4) * P])
                    wc4 = stream.tile([P, DC, 2, 4 * P], F8, tag="wmTc",
                                      name="wtc", bufs=2)
                    for a in range(2):
                        nc.sync.dma_start(
                            wc4[:, :, a, :],
                            wmT8c_v[:, :, a, ho * P:(ho + 4) * P])
                    mlp1_emit.w4, mlp1_emit.wc4 = w4, wc4
                o4 = (ho % 4) * P
                wt = mlp1_emit.w4[:, :, o4:o4 + P]
                wtc = mlp1_emit.wc4[:, :, :, o4:o4 + P]

                def chain(ns, ho=ho, wt=wt, wtc=wtc):
                    pt = ps_atq()[:, 0:256]
                    nsl = slice(ns * 256, (ns + 1) * 256)
                    for cp in range(DC // 2):
                        nc.tensor.matmul(
                            pt[:], wt[:, 2 * cp:2 * cp + 2, :],
                            xT8c[:, 2 * cp:2 * cp + 2, 0, nsl],
                            start=(cp == 0), stop=False, perf_mode=DR,
                        )
                    for c in range(DC):
                        nc.tensor.matmul(
                            pt[:], wtc[:, c, :, :], xT8c[:, c, :, nsl],
                            start=False, stop=(c == DC - 1), perf_mode=DR,
                        )
                    nc.vector.tensor_scalar_max(
                        hid8[:, ho, nsl], pt[:], 0.0)
                return [lambda ns=ns: chain(ns) for ns in range(4)]

            # ---- per-head attention pieces ----
            def s_exp(h, filler):
                """E8 = exp(beta/SW^2 * S) in fp8, S via zero-padded DR."""
                zo = (h % 2) * Z
                c = h // 2
                E8 = hd.tile([P, NC, N], F8, tag="E8", name="E8", bufs=2)
                for qo in range(NC):
                    pt = psE()
                    for j in range(4):
                        nc.tensor.matmul(
                            pt[:, j * 256:(j + 1) * 256],
                            QT8[zo:zo + Z, c, :, qo * P:(qo + 1) * P],
                            KT8v[zo:zo + Z, c * N + j * 256:c * N + j * 256 + 512]
                            .rearrange("p (a b) -> p a b", a=2),
                            start=True, stop=True, perf_mode=DR,
                        )
                    nc.scalar.activation(
                        E8[:, qo, :], pt[:], Exp, scale=betap[:, h:h + 1])
                    f = next(filler, None)
                    if f is not None:
                        f()
                return E8

            def et_xbar(h, E8):
                ET8 = hd.tile([P, 32, P], U16, tag="ET8", name="ET8", bufs=2)
                nc.sync.dma_start(
                    ET8[:], E8[:].bitcast(U16).rearrange("p a b -> p (a b)"),
                    transpose=True)
                return ET8

            def et_lhs(ET8, qo, kcp, s):
                # A^T stationary: [128 kp, 2 (kc-pair, 256B stride), 128 q
                # (step 2, parity s)] — dual-fp8 LdWeights needs the pair
                # stride 16B-aligned, so the byte-interleave is consumed as
                # two parity lanes instead.
                sl = ET8[:].bitcast(F8)[:, qo * 4 + 2 * kcp:qo * 4 + 2 * kcp + 2, :]
                return sl.rearrange("p a (f s) -> p a s f", s=2)[:, :, s, :]

            def rowsum(h, ET8):
                """r[q] per qo via tiny PE ones-matmuls on packed A^T."""
                pr = ps_atq()[:, 256:256 + NC]
                for qo in range(NC):
                    for i, (kcp, s) in enumerate(
                            ((0, 0), (0, 1), (1, 0), (1, 1))):
                        nc.tensor.matmul(
                            pr[:, qo:qo + 1], et_lhs(ET8, qo, kcp, s),
                            ones8[:, :, 0:1],
                            start=(i == 0), stop=(i == 3), perf_mode=DR,
                        )
                return pr

            def make_qs(h, pr):
                """rc = C/r ; Qs[z-half of head h] = Qn * rc (fp8)."""
                rc = hd.tile([P, NC], F32, tag="rc", name="rc", bufs=2)
                zo = (h % 2) * Z
                nc.vector.reciprocal(rc[:], pr[:])
                for qo in range(NC):
                    nc.vector.tensor_scalar(
                        Qs8[:, qo, zo:zo + Z], Qn8[:, qo, h * Z:(h + 1) * Z],
                        rc[:, qo:qo + 1], C, Mult, Mult)
                return rc

            def atq_direct(h, E8):
                """ATQT[z,k] = sum_q Qs[q,z] E8[q,k], evicted per kseg.
                The stationary spans all 128 Qs columns; the other head's
                64 rows compute junk that the evict skips."""
                zo = (h % 2) * Z
                c = h // 2
                for ks in range(4):
                    pt = ps_atq()[:, 0:256]
                    for qp in range(4):
                        nc.tensor.matmul(
                            pt[:],
                            Qs8[:, 2 * qp:2 * qp + 2, :],
                            E8[:, 2 * qp:2 * qp + 2, ks * 256:(ks + 1) * 256],
                            start=(qp == 0), stop=(qp == 3), perf_mode=DR,
                        )
                    nc.vector.tensor_copy(
                        ATQT8[zo:zo + Z, c, 2 * ks:2 * ks + 2, :],
                        pt[zo:zo + Z, :].rearrange("p (a b) -> p a b", b=P))

            def av_part(h, ET8):
                """AV[q,z] = sum_k A^T[k,q] K[k,z] via packed ET8 + Knp."""
                c = h // 2
                if h % 2 == 0:
                    av_part.pav_e = ps_pav()
                    pav = av_part.pav_e
                else:
                    av_part.pav_o = ps_pav()
                    pav = av_part.pav_o
                Knpf = Knp[:].bitcast(F8)   # [128, 6, 4, 256]
                for qo in range(NC):
                    for i, (kcp, s) in enumerate(
                            ((0, 0), (0, 1), (1, 0), (1, 1))):
                        rhs = Knpf[:, c, 2 * kcp:2 * kcp + 2, :].rearrange(
                            "p a (f s) -> p a s f", s=2)[
                                :, :, s, (h % 2) * Z:(h % 2) * Z + Z]
                        nc.tensor.matmul(
                            pav[:, qo, :], et_lhs(ET8, qo, kcp, s), rhs,
                            start=(i == 0), stop=(i == 3), perf_mode=DR,
                        )

            def av_evict(hpair, rc_e, rc_o):
                """An = pav * (C/r) per head, PE-transpose into AVT8."""
                c = hpair
                An8 = hd.tile([P, NC, 2 * Z], F8, tag="An8", name="An8", bufs=2)
                for qo in range(NC):
                    nc.vector.tensor_scalar(
                        An8[:, qo, 0:Z], av_part.pav_e[:, qo, :],
                        rc_e[:, qo:qo + 1], C, Mult, Mult)
                    nc.vector.tensor_scalar(
                        An8[:, qo, Z:2 * Z], av_part.pav_o[:, qo, :],
                        rc_o[:, qo:qo + 1], C, Mult, Mult)
                ptb = psE()[:].bitcast(F8)
                ptv = ptb[:, 0:NC * 2 * P].rearrange(
                    "p (a f s) -> p a s f", a=NC, s=2)   # [128, 8, 2, 128]
                for qo in range(NC):
                    nc.tensor.transpose(
                        ptv[:, qo, 0, :], An8[:, qo, :], ident8[:])
                nc.vector.tensor_copy(AVT8[:, c, :, :], ptv[:, :, 0, :])

            # ---- software pipeline over heads ----
            # phase h: S(h)+exp(h) with mlp1 fillers; then rowsum/Qs/ATQ/AV
            # for h-1 (its XBAR landed during exp(h)); pair-evict at odd h-1.
            mlp_chunks = [mlp1_emit(ho) for ho in range(2)]
            # Knp XBARs wait on the KT8 evicts; emitted after the first mlp
            # block loads so those aren't head-of-line blocked on SP.
            KT8u = KT8[:].bitcast(U16)       # [128, 3200]
            for c in range(DC):
                nc.sync.dma_start(
                    Knp[:, c, :, :],
                    KT8u[:, c * 512:(c + 1) * 512], transpose=True)
            prev = None           # (E8, ET8) of h-1
            rcs = {}
            next_ho = 2
            for h in range(H + 1):
                filler_items = []
                if h < H:
                    # two mlp1 chunks (8 chains) per phase: one chain per qo
                    for _ in range(2):
                        if mlp_chunks:
                            filler_items.extend(mlp_chunks.pop(0))
                        if next_ho < HC:
                            mlp_chunks.append(mlp1_emit(next_ho))
                            next_ho += 1
                    filler = iter(filler_items)
                    E8 = s_exp(h, filler)
                    for f in filler:
                        f()
                if prev is not None:
                    hp = h - 1
                    pE8, pET8 = prev
                    pr = rowsum(hp, pET8)
                    rcs[hp] = make_qs(hp, pr)
                    atq_direct(hp, pE8)
                    av_part(hp, pET8)
                    if hp % 2 == 1:
                        av_evict(hp // 2, rcs[hp - 1], rcs[hp])
                        if dbg and hp == 1:
                            nc.sync.dma_start(dbg_d["r0"].ap(), rcs[0][:])
                if h < H:
                    ET8 = et_xbar(h, E8)
                    prev = (E8, ET8)
                    if dbg and h == 0:
                        nc.sync.dma_start(dbg_d["E80"].ap(), E8[:])
                        nc.sync.dma_start(dbg_d["ET80"].ap(), ET8[:])
                if h == 7:
                    nc.sync.dma_start(wq8[:], wq8_v)
                if h == 8:
                    nc.sync.dma_start(wk8[:], wk8_v)
                if h == 9:
                    nc.sync.dma_start(wm8b[:], wm8b_v)
                if h == 10:
                    nc.sync.dma_start(wm8c[:], wm8c_v)

            if dbg:
                nc.sync.dma_start(dbg_d["QT8"].ap(), QT8[:])
                nc.sync.dma_start(dbg_d["KT8"].ap(), KT8[:])
                nc.sync.dma_start(dbg_d["Qn8"].ap(), Qn8[:])
                nc.sync.dma_start(dbg_d["Knp"].ap(), Knp[:])
                nc.sync.dma_start(dbg_d["hid8"].ap(), hid8[:])
                nc.sync.dma_start(dbg_d["AVT8"].ap(), AVT8[:])
                nc.sync.dma_start(dbg_d["ATQT8"].ap(), ATQT8[:])

            # ---- stage 4: out = (AVc@Wq + ATQc@Wk + hid@(Wm*C)) / (SW^2 C)
            OSC = 1.0 / (SW * SW * C)
            for rnd in range(4):
                pouts = [psE() for _ in range(2)]
                for i, po in enumerate(pouts):
                    no = 2 * rnd + i
                    pt = po[:, 0:D]
                    for ds in range(3):
                        dsl = slice(ds * 256, (ds + 1) * 256)
                        for cp in range(DC // 2):
                            for lhsT, w in ((AVT8, wq8), (ATQT8, wk8)):
                                nc.tensor.matmul(
                                    pt[:, dsl],
                                    lhsT[:, 2 * cp:2 * cp + 2, no, :],
                                    w[:, 2 * cp:2 * cp + 2, dsl],
                                    start=(cp == 0 and lhsT is AVT8),
                                    stop=False, perf_mode=DR,
                                )
                        for wm_t in (wm8b, wm8c):
                            for hp in range(HC // 2):
                                nc.tensor.matmul(
                                    pt[:, dsl],
                                    hid8[:, 2 * hp:2 * hp + 2, no * P:(no + 1) * P],
                                    wm_t[:, 2 * hp:2 * hp + 2, dsl],
                                    start=False,
                                    stop=(wm_t is wm8c and hp == HC // 2 - 1),
                                    perf_mode=DR,
                                )
                osb = stream.tile([P, 2, D], BF, tag="osb", name="osb", bufs=2)
                for i in range(2):
                    nc.scalar.activation(
                        osb[:, i, :], pouts[i][:, 0:D],
                        mybir.ActivationFunctionType.Copy, scale=OSC)
                nc.sync.dma_start(out_v[:, 2 * rnd:2 * rnd + 2, :], osb[:])

    nc.compile()
    return nc


def _prep(x, Wq, Wk, betas, W_mlp):
    x = np.asarray(x, dtype=np.float32)
    Wq = np.asarray(Wq, dtype=np.float32)
    Wk = np.asarray(Wk, dtype=np.float32)
    betas = np.asarray(betas, dtype=np.float32)
    W_mlp = np.asarray(W_mlp, dtype=np.float32)

    wqT8 = np.ascontiguousarray(Wq.T * SW).astype(NPF8)
    wkT8 = np.ascontiguousarray(Wk.T * SW).astype(NPF8)
    wq8 = np.ascontiguousarray(Wq * SW).astype(NPF8)
    wk8 = np.ascontiguousarray(Wk * SW).astype(NPF8)
    wmT8 = np.ascontiguousarray(W_mlp.T * SW).astype(NPF8)
    wmT8c = np.empty((D, 2, HID), NPF8)
    wmT8c[:, 0, :] = (W_mlp.T * SW - wmT8.astype(np.float32)).astype(NPF8)
    wmT8c[:, 1, :] = (wmT8.astype(np.float32) / 16.0).astype(NPF8)
    wm8b = np.ascontiguousarray(W_mlp * (SW * C)).astype(NPF8)
    wm8c = np.ascontiguousarray(
        W_mlp * (SW * C) - wm8b.astype(np.float32)).astype(NPF8)
    betap = np.ascontiguousarray(np.broadcast_to(
        (betas / (SW * SW))[None, :], (P, H))).astype(np.float32)
    ident = np.eye(P, dtype=np.float32).astype(NPBF)
    ident8 = np.eye(P, dtype=np.float32).astype(NPF8)

    in_maps = []
    for b in range(B):
        xT = np.ascontiguousarray(x[b].T)
        x8 = xT.astype(NPF8)
        xc = np.empty((D, 2, N), NPF8)
        xc[:, 0, :] = x8
        xc[:, 1, :] = (16.0 * (xT - x8.astype(np.float32))).astype(NPF8)
        in_maps.append({
            "xT8c": xc,
            "wqT8": wqT8, "wkT8": wkT8, "wq8": wq8, "wk8": wk8,
            "wmT8": wmT8, "wmT8c": wmT8c, "wm8b": wm8b, "wm8c": wm8c,
            "betap": betap, "ident": ident, "ident8": ident8,
        })
    return in_maps


def kernel(x, Wq, Wk, betas, W_mlp, _trace=False, _dbg=False):
    key = "nc_dbg" if _dbg else "nc"
    if key not in _CACHE:
        _CACHE[key] = _build(dbg=_dbg)
    nc = _CACHE[key]
    in_maps = _prep(x, Wq, Wk, betas, W_mlp)
    core_ids = list(range(B)) if not _dbg else [0]
    res = run_bass_kernel_spmd(nc, in_maps[:len(core_ids)], core_ids=core_ids,
                               trace=_trace)
    _CACHE["last_result"] = res
    if _dbg:
        return res
    out = np.stack([res.results[b]["out"] for b in range(B)], axis=0)
    return out.astype(np.float32)


# revision 44
# speedup vs baseline: 1.1869x; 1.0028x over previous
"""KQEnergyBlock Trainium2 Bass kernel, v3 (fp8 DoubleRow redesign).

Math per batch element (see reference):
  Q = x Wq^T, K = x Wk^T            (N, D), heads = 64-col slices
  S_h = beta_h Q_h K_h^T ; A_h = softmax(S_h)
  T1 = AVc @ Wq   (AVc  = concat_h A_h K_h)
  T2 = ATQc @ Wk  (ATQc = concat_h A_h^T Q_h)
  out = T1 + T2 + relu(x Wm^T) Wm

Cost-model facts driving the design (TimelineSim):
  - matmul engine time = out_free_size * cycles_per_row; fp8e4 (e4m3) with
    perf_mode=DoubleRow costs 0.5 cycles/row while contracting [K,2,*]
    operand pairs -> 4x bf16 MAC throughput when pairs carry real data.
  - ACT exp costs ~1038ns per [128,1024] instruction: the 12 M softmax
    exps (~100us) are the floor; everything else overlaps around it.
  - XBAR DMA transpose costs 14ns per 16x128 tile and requires a 2-byte
    dtype: transposing fp8 PAIRS bitcast as uint16 halves the tile count
    AND lands A^T pre-paired in exactly the [K,2,M] layout DoubleRow wants.

Numerics (threshold 2e-2; measured ~1.3e-2 in simulation):
  - mlp is ~96% of output norm, so mlp1 runs in bf16 (kills x- and W1-
    quantization error, and bf16 needs no DR pairs so it is cheaper than a
    compensated fp8 path).
  - stage4 hid@Wm runs fp8-DR with a host-prepared residual-weight pass
    (wm8c = fp8(SW*C*Wm - wm8b)) accumulated into the same psum.
  - weights scaled by SW=32 (keeps everything under fp8e4m3's 240 max),
    softmax renorm constant C=16 folded into Qs / AV-evict / final evict.

Sharding: data-parallel over batch B=8, one element per core, no
collectives.
"""

import numpy as np
import ml_dtypes

import concourse.mybir as mybir
import concourse.tile as tile
from concourse import bacc
from concourse.bass_utils import run_bass_kernel_spmd

B, N, D = 8, 1024, 768
H, Z = 12, 64
HID = 3072
P = 128
DC = D // P      # 6
NC = N // P      # 8
HC = HID // P    # 24
BF = mybir.dt.bfloat16
F32 = mybir.dt.float32
F8 = mybir.dt.float8e4
U16 = mybir.dt.uint16
Exp = mybir.ActivationFunctionType.Exp
DR = mybir.MatmulPerfMode.DoubleRow
Mult = mybir.AluOpType.mult

NPBF = ml_dtypes.bfloat16
NPF8 = ml_dtypes.float8_e4m3

SW = 32.0        # weight scale into fp8
C = 16.0         # softmax renorm constant

_CACHE = {}


def _build(dbg=False):
    nc = bacc.Bacc("TRN2", target_bir_lowering=False, debug=False, num_devices=8)
    dbg_d = {}
    if dbg:
        for nm, shp, dt in (("QT8", [P, DC, 2, N], F8), ("KT8", [P, DC * N + 256], F8),
                            ("E80", [P, NC, N], F8), ("ET80", [P, 32, P], U16),
                            ("Qn8", [P, NC, D], F8), ("Knp", [P, DC, 4, P], U16),
                            ("hid8", [P, HC, N], F8), ("AVT8", [P, DC, NC, P], F8),
                            ("ATQT8", [P, DC, NC, P], F8), ("r0", [P, NC], F32)):
            dbg_d[nm] = nc.dram_tensor("dbg_" + nm, shp, dt, kind="ExternalOutput")

    xT8c_d = nc.dram_tensor("xT8c", [D, 2, N], F8, kind="ExternalInput")
    wqT8_d = nc.dram_tensor("wqT8", [D, D], F8, kind="ExternalInput")
    wkT8_d = nc.dram_tensor("wkT8", [D, D], F8, kind="ExternalInput")
    wq8_d = nc.dram_tensor("wq8", [D, D], F8, kind="ExternalInput")
    wk8_d = nc.dram_tensor("wk8", [D, D], F8, kind="ExternalInput")
    wmT8_d = nc.dram_tensor("wmT8", [D, HID], F8, kind="ExternalInput")
    wmT8c_d = nc.dram_tensor("wmT8c", [D, 2, HID], F8, kind="ExternalInput")
    wm8b_d = nc.dram_tensor("wm8b", [HID, D], F8, kind="ExternalInput")
    wm8c_d = nc.dram_tensor("wm8c", [HID, D], F8, kind="ExternalInput")
    betap_d = nc.dram_tensor("betap", [P, H], F32, kind="ExternalInput")
    ident_d = nc.dram_tensor("ident", [P, P], BF, kind="ExternalInput")
    ident8_d = nc.dram_tensor("ident8", [P, P], F8, kind="ExternalInput")
    out_d = nc.dram_tensor("out", [N, D], BF, kind="ExternalOutput")

    xT8c_v = xT8c_d.ap().rearrange("(c p) a n -> p c a n", p=P)  # [128,6,2,1024]
    wqT8_v = wqT8_d.ap().rearrange("(c p) e -> p c e", p=P)
    wkT8_v = wkT8_d.ap().rearrange("(c p) e -> p c e", p=P)
    wq8_v = wq8_d.ap().rearrange("(c p) d -> p c d", p=P)
    wk8_v = wk8_d.ap().rearrange("(c p) d -> p c d", p=P)
    wmT8_v = wmT8_d.ap().rearrange("(c p) h -> p c h", p=P)   # [128, 6, 3072]
    wmT8c_v = wmT8c_d.ap().rearrange("(c p) a h -> p c a h", p=P)
    wm8b_v = wm8b_d.ap().rearrange("(c p) d -> p c d", p=P)   # [128, 24, 768]
    wm8c_v = wm8c_d.ap().rearrange("(c p) d -> p c d", p=P)
    out_v = out_d.ap().rearrange("(c p) d -> p c d", p=P)     # [128, 8, 768]

    with tile.TileContext(nc) as tc:
        with (
            tc.tile_pool(name="acts", bufs=1) as acts,
            tc.tile_pool(name="hd", bufs=1) as hd,
            tc.tile_pool(name="stream", bufs=1) as stream,
            tc.tile_pool(name="ps", bufs=1, space="PSUM") as ps,
        ):
            # ---- persistent tiles ----
            xT8c = acts.tile([P, DC, 2, N], F8)
            wqT8 = acts.tile([P, DC, D], F8)
            wkT8 = acts.tile([P, DC, D], F8)
            wq8 = acts.tile([P, DC, D], F8)
            wk8 = acts.tile([P, DC, D], F8)
            wm8b = acts.tile([P, HC, D], F8)
            wm8c = acts.tile([P, HC, D], F8)
            betap = acts.tile([P, H], F32)
            ident = acts.tile([P, P], BF)
            ident8 = acts.tile([P, P], F8)
            ones8 = acts.tile([P, 2, 16], F8)
            # projections: QT8 slot1 is a zero pad (DoubleRow partner row);
            # KT8 gets a 256-col zero tail so the k-window rhs can overrun.
            QT8 = acts.tile([P, DC, 2, N], F8)
            KT8 = acts.tile([P, DC * N + 256], F8)
            Qn8 = acts.tile([P, NC, D], F8)
            Knp = acts.tile([P, DC, 4, P], U16)     # paired Kn from XBAR
            hid8 = acts.tile([P, HC, N], F8)
            # Qs: persistent [q, qo, 128]; head h writes z-cols
            # [(h%2)*64, +64) so consecutive heads use disjoint halves and
            # the ATQ stationary can span all 128 columns (dual-fp8 matmuls
            # must write psum starting at partition 0).
            Qs8 = acts.tile([P, NC, P], F8)
            AVT8 = acts.tile([P, DC, NC, P], F8)
            ATQT8 = acts.tile([P, DC, NC, P], F8)

            def psE():
                return ps.tile([P, N], F32, tag="psE", name="pt", bufs=2)

            def ps_pav():
                return ps.tile([P, NC, Z], F32, tag="pav", name="pav", bufs=2)

            def ps_atq():
                # [:, 0:256] = ATQT kseg accumulator; [:, 256:264] doubles as
                # the rowsum accumulator (separate allocations, same ring).
                return ps.tile([P, 264], F32, tag="patq", name="patq", bufs=2)

            # ---- input loads (ordered for proj critical path) ----
            # wait-free input loads ride the ACT HWDGE queue: they dispatch
            # immediately (nothing to wait on) and keep the SP queue clear
            # for the data-dependent XBAR transposes of the early heads.
            nc.scalar.dma_start(wqT8[:], wqT8_v)
            nc.scalar.dma_start(xT8c[:, :, 0, :], xT8c_v[:, :, 0, :])
            nc.scalar.dma_start(wkT8[:], wkT8_v)
            nc.scalar.dma_start(xT8c[:, :, 1, :], xT8c_v[:, :, 1, :])
            nc.scalar.dma_start(betap[:], betap_d.ap())
            nc.scalar.dma_start(ident[:], ident_d.ap())
            nc.scalar.dma_start(ident8[:], ident8_d.ap())
            nc.gpsimd.memset(QT8[:, :, 1, :], 0.0)
            nc.gpsimd.memset(KT8[:, DC * N:], 0.0)
            nc.vector.memset(ones8[:], 1.0)
            nc.gpsimd.memset(Qs8[:, :, Z:], 0.0)

            KT8v = KT8[:]  # [128, 6400]

            # ---- projections: QT/KT feature-major, fp8 DoubleRow ----
            for eo in range(DC):
                for wT, isq in ((wqT8, True), (wkT8, False)):
                    pt = psE()
                    # segment-major: a region's full start->stop accumulation
                    # chain must complete before the next start in the same
                    # 2KB psum zero-region
                    for ns in range(4):
                        for cp in range(DC // 2):
                            nc.tensor.matmul(
                                pt[:, ns * 256:(ns + 1) * 256],
                                wT[:, 2 * cp:2 * cp + 2, eo * P:(eo + 1) * P],
                                xT8c[:, 2 * cp:2 * cp + 2, 0,
                                     ns * 256:(ns + 1) * 256],
                                start=(cp == 0), stop=(cp == DC // 2 - 1),
                                perf_mode=DR,
                            )
                    if isq:
                        nc.vector.tensor_copy(QT8[:, eo, 0, :], pt[:])
                    else:
                        nc.vector.tensor_copy(
                            KT8v[:, eo * N:(eo + 1) * N], pt[:])

            # Qn: natural-layout Q via PE transposes (fp8). FP8 transpose
            # writes with element step 2; evict reads the even-byte lanes.
            for eo in range(DC):
                ptb = psE()[:].bitcast(F8)   # [128, 4096]
                ptv = ptb[:, 0:NC * 2 * P].rearrange(
                    "p (a f s) -> p a s f", a=NC, s=2)   # [128, 8, 2, 128]
                for qo in range(NC):
                    nc.tensor.transpose(
                        ptv[:, qo, 0, :],
                        QT8[:, eo, 0, qo * P:(qo + 1) * P], ident8[:])
                nc.vector.tensor_copy(
                    Qn8[:, :, eo * P:(eo + 1) * P], ptv[:, :, 0, :])

            # ---- mlp1 (fp8 DR): hid8[ho] = relu(Wm x^T) in four 256-wide
            # chains per ho on the patq psum ring. Each chain: 3 main
            # DoubleRow instrs (c-pairs) + 6 correction instrs whose DR
            # slots carry (dW1[c], W1/16[c]) x (x8[c], 16*dx[c]) — removing
            # both the weight- and activation-quantization error of the
            # main pass at half of bf16's cycle cost.
            def mlp1_emit(ho):
                if ho % 4 == 0:
                    w4 = stream.tile([P, DC, 4 * P], F8, tag="wmT", name="wt",
                                     bufs=2)
                    nc.sync.dma_start(
                        w4[:], wmT8_v[:, :, ho * P:(ho + 4) * P])
                    wc4 = stream.tile([P, DC, 2, 4 * P], F8, tag="wmTc",
                                      name="wtc", bufs=2)
                    for a in range(2):
                        nc.sync.dma_start(
                            wc4[:, :, a, :],
                            wmT8c_v[:, :, a, ho * P:(ho + 4) * P])
                    mlp1_emit.w4, mlp1_emit.wc4 = w4, wc4
                o4 = (ho % 4) * P
                wt = mlp1_emit.w4[:, :, o4:o4 + P]
                wtc = mlp1_emit.wc4[:, :, :, o4:o4 + P]

                def chain_a(ns, cell, ho=ho, wt=wt):
                    cell[0] = ps_atq()[:, 0:256]
                    nsl = slice(ns * 256, (ns + 1) * 256)
                    for cp in range(DC // 2):
                        nc.tensor.matmul(
                            cell[0][:], wt[:, 2 * cp:2 * cp + 2, :],
                            xT8c[:, 2 * cp:2 * cp + 2, 0, nsl],
                            start=(cp == 0), stop=False, perf_mode=DR,
                        )

                def chain_b(ns, cell, ho=ho, wtc=wtc):
                    nsl = slice(ns * 256, (ns + 1) * 256)
                    for c in range(DC):
                        nc.tensor.matmul(
                            cell[0][:], wtc[:, c, :, :], xT8c[:, c, :, nsl],
                            start=False, stop=(c == DC - 1), perf_mode=DR,
                        )
                    nc.vector.tensor_scalar_max(
                        hid8[:, ho, nsl], cell[0][:], 0.0)
                out = []
                for ns in range(4):
                    cell = [None]
                    out.append(lambda ns=ns, cell=cell: chain_a(ns, cell))
                    out.append(lambda ns=ns, cell=cell: chain_b(ns, cell))
                return out

            # ---- per-head attention pieces ----
            def s_exp(h, filler):
                """E8 = exp(beta/SW^2 * S) in fp8, S via zero-padded DR."""
                zo = (h % 2) * Z
                c = h // 2
                E8 = hd.tile([P, NC, N], F8, tag="E8", name="E8", bufs=2)
                for qo in range(NC):
                    pt = psE()
                    for j in range(4):
                        nc.tensor.matmul(
                            pt[:, j * 256:(j + 1) * 256],
                            QT8[zo:zo + Z, c, :, qo * P:(qo + 1) * P],
                            KT8v[zo:zo + Z, c * N + j * 256:c * N + j * 256 + 512]
                            .rearrange("p (a b) -> p a b", a=2),
                            start=True, stop=True, perf_mode=DR,
                        )
                    nc.scalar.activation(
                        E8[:, qo, :], pt[:], Exp, scale=betap[:, h:h + 1])
                    f = next(filler, None)
                    if f is not None:
                        f()
                return E8

            def et_xbar(h, E8):
                ET8 = hd.tile([P, 32, P], U16, tag="ET8", name="ET8", bufs=2)
                nc.sync.dma_start(
                    ET8[:], E8[:].bitcast(U16).rearrange("p a b -> p (a b)"),
                    transpose=True)
                return ET8

            def et_lhs(ET8, qo, kcp, s):
                # A^T stationary: [128 kp, 2 (kc-pair, 256B stride), 128 q
                # (step 2, parity s)] — dual-fp8 LdWeights needs the pair
                # stride 16B-aligned, so the byte-interleave is consumed as
                # two parity lanes instead.
                sl = ET8[:].bitcast(F8)[:, qo * 4 + 2 * kcp:qo * 4 + 2 * kcp + 2, :]
                return sl.rearrange("p a (f s) -> p a s f", s=2)[:, :, s, :]

            def rowsum(h, ET8):
                """r[q] per qo via tiny PE ones-matmuls on packed A^T."""
                pr = ps_atq()[:, 256:256 + NC]
                for qo in range(NC):
                    for i, (kcp, s) in enumerate(
                            ((0, 0), (0, 1), (1, 0), (1, 1))):
                        nc.tensor.matmul(
                            pr[:, qo:qo + 1], et_lhs(ET8, qo, kcp, s),
                            ones8[:, :, 0:1],
                            start=(i == 0), stop=(i == 3), perf_mode=DR,
                        )
                return pr

            def make_qs(h, pr):
                """rc = C/r ; Qs[z-half of head h] = Qn * rc (fp8)."""
                rc = hd.tile([P, NC], F32, tag="rc", name="rc", bufs=2)
                zo = (h % 2) * Z
                nc.vector.reciprocal(rc[:], pr[:])
                for qo in range(NC):
                    nc.vector.tensor_scalar(
                        Qs8[:, qo, zo:zo + Z], Qn8[:, qo, h * Z:(h + 1) * Z],
                        rc[:, qo:qo + 1], C, Mult, Mult)
                return rc

            def atq_direct(h, E8):
                """ATQT[z,k] = sum_q Qs[q,z] E8[q,k], evicted per kseg.
                The stationary spans all 128 Qs columns; the other head's
                64 rows compute junk that the evict skips."""
                zo = (h % 2) * Z
                c = h // 2
                for ks in range(4):
                    pt = ps_atq()[:, 0:256]
                    for qp in range(4):
                        nc.tensor.matmul(
                            pt[:],
                            Qs8[:, 2 * qp:2 * qp + 2, :],
                            E8[:, 2 * qp:2 * qp + 2, ks * 256:(ks + 1) * 256],
                            start=(qp == 0), stop=(qp == 3), perf_mode=DR,
                        )
                    nc.vector.tensor_copy(
                        ATQT8[zo:zo + Z, c, 2 * ks:2 * ks + 2, :],
                        pt[zo:zo + Z, :].rearrange("p (a b) -> p a b", b=P))

            def av_part(h, ET8):
                """AV[q,z] = sum_k A^T[k,q] K[k,z] via packed ET8 + Knp."""
                c = h // 2
                if h % 2 == 0:
                    av_part.pav_e = ps_pav()
                    pav = av_part.pav_e
                else:
                    av_part.pav_o = ps_pav()
                    pav = av_part.pav_o
                Knpf = Knp[:].bitcast(F8)   # [128, 6, 4, 256]
                for qo in range(NC):
                    for i, (kcp, s) in enumerate(
                            ((0, 0), (0, 1), (1, 0), (1, 1))):
                        rhs = Knpf[:, c, 2 * kcp:2 * kcp + 2, :].rearrange(
                            "p a (f s) -> p a s f", s=2)[
                                :, :, s, (h % 2) * Z:(h % 2) * Z + Z]
                        nc.tensor.matmul(
                            pav[:, qo, :], et_lhs(ET8, qo, kcp, s), rhs,
                            start=(i == 0), stop=(i == 3), perf_mode=DR,
                        )

            def av_evict(hpair, rc_e, rc_o):
                """An = pav * (C/r) per head, PE-transpose into AVT8."""
                c = hpair
                An8 = hd.tile([P, NC, 2 * Z], F8, tag="An8", name="An8", bufs=2)
                for qo in range(NC):
                    nc.vector.tensor_scalar(
                        An8[:, qo, 0:Z], av_part.pav_e[:, qo, :],
                        rc_e[:, qo:qo + 1], C, Mult, Mult)
                    nc.vector.tensor_scalar(
                        An8[:, qo, Z:2 * Z], av_part.pav_o[:, qo, :],
                        rc_o[:, qo:qo + 1], C, Mult, Mult)
                ptb = psE()[:].bitcast(F8)
                ptv = ptb[:, 0:NC * 2 * P].rearrange(
                    "p (a f s) -> p a s f", a=NC, s=2)   # [128, 8, 2, 128]
                for qo in range(NC):
                    nc.tensor.transpose(
                        ptv[:, qo, 0, :], An8[:, qo, :], ident8[:])
                nc.vector.tensor_copy(AVT8[:, c, :, :], ptv[:, :, 0, :])

            # ---- software pipeline over heads ----
            # phase h: S(h)+exp(h) with mlp1 fillers; then rowsum/Qs/ATQ/AV
            # for h-1 (its XBAR landed during exp(h)); pair-evict at odd h-1.
            mlp_chunks = [mlp1_emit(ho) for ho in range(2)]
            # Knp XBARs wait on the KT8 evicts; emitted after the first mlp
            # block loads so those aren't head-of-line blocked on SP.
            KT8u = KT8[:].bitcast(U16)       # [128, 3200]
            for c in range(DC):
                nc.sync.dma_start(
                    Knp[:, c, :, :],
                    KT8u[:, c * 512:(c + 1) * 512], transpose=True)
            prev = None           # (E8, ET8) of h-1
            rcs = {}
            next_ho = 2
            for h in range(H + 1):
                filler_items = []
                if h < H:
                    # two mlp1 chunks (8 chains) per phase: one chain per qo
                    for _ in range(2):
                        if mlp_chunks:
                            filler_items.extend(mlp_chunks.pop(0))
                        if next_ho < HC:
                            mlp_chunks.append(mlp1_emit(next_ho))
                            next_ho += 1
                    filler = iter(filler_items)
                    E8 = s_exp(h, filler)
                    for f in filler:
                        f()
                if prev is not None:
                    hp = h - 1
                    pE8, pET8 = prev
                    pr = rowsum(hp, pET8)
                    rcs[hp] = make_qs(hp, pr)
                    atq_direct(hp, pE8)
                    av_part(hp, pET8)
                    if hp % 2 == 1:
                        av_evict(hp // 2, rcs[hp - 1], rcs[hp])
                        if dbg and hp == 1:
                            nc.sync.dma_start(dbg_d["r0"].ap(), rcs[0][:])
                if h < H:
                    ET8 = et_xbar(h, E8)
                    prev = (E8, ET8)
                    if dbg and h == 0:
                        nc.sync.dma_start(dbg_d["E80"].ap(), E8[:])
                        nc.sync.dma_start(dbg_d["ET80"].ap(), ET8[:])
                if h == 7:
                    nc.sync.dma_start(wq8[:], wq8_v)
                if h == 8:
                    nc.sync.dma_start(wk8[:], wk8_v)
                if h == 9:
                    nc.sync.dma_start(wm8b[:], wm8b_v)
                if h == 10:
                    nc.sync.dma_start(wm8c[:], wm8c_v)

            if dbg:
                nc.sync.dma_start(dbg_d["QT8"].ap(), QT8[:])
                nc.sync.dma_start(dbg_d["KT8"].ap(), KT8[:])
                nc.sync.dma_start(dbg_d["Qn8"].ap(), Qn8[:])
                nc.sync.dma_start(dbg_d["Knp"].ap(), Knp[:])
                nc.sync.dma_start(dbg_d["hid8"].ap(), hid8[:])
                nc.sync.dma_start(dbg_d["AVT8"].ap(), AVT8[:])
                nc.sync.dma_start(dbg_d["ATQT8"].ap(), ATQT8[:])

            # ---- stage 4: out = (AVc@Wq + ATQc@Wk + hid@(Wm*C)) / (SW^2 C)
            OSC = 1.0 / (SW * SW * C)
            for rnd in range(4):
                pouts = [psE() for _ in range(2)]
                for i, po in enumerate(pouts):
                    no = 2 * rnd + i
                    pt = po[:, 0:D]
                    for ds in range(3):
                        dsl = slice(ds * 256, (ds + 1) * 256)
                        for cp in range(DC // 2):
                            for lhsT, w in ((AVT8, wq8), (ATQT8, wk8)):
                                nc.tensor.matmul(
                                    pt[:, dsl],
                                    lhsT[:, 2 * cp:2 * cp + 2, no, :],
                                    w[:, 2 * cp:2 * cp + 2, dsl],
                                    start=(cp == 0 and lhsT is AVT8),
                                    stop=False, perf_mode=DR,
                                )
                        for wm_t in (wm8b, wm8c):
                            for hp in range(HC // 2):
                                nc.tensor.matmul(
                                    pt[:, dsl],
                                    hid8[:, 2 * hp:2 * hp + 2, no * P:(no + 1) * P],
                                    wm_t[:, 2 * hp:2 * hp + 2, dsl],
                                    start=False,
                                    stop=(wm_t is wm8c and hp == HC // 2 - 1),
                                    perf_mode=DR,
                                )
                osb = stream.tile([P, 2, D], BF, tag="osb", name="osb", bufs=2)
                for i in range(2):
                    nc.scalar.activation(
                        osb[:, i, :], pouts[i][:, 0:D],
                        mybir.ActivationFunctionType.Copy, scale=OSC)
                nc.sync.dma_start(out_v[:, 2 * rnd:2 * rnd + 2, :], osb[:])

    nc.compile()
    return nc


def _prep(x, Wq, Wk, betas, W_mlp):
    x = np.asarray(x, dtype=np.float32)
    Wq = np.asarray(Wq, dtype=np.float32)
    Wk = np.asarray(Wk, dtype=np.float32)
    betas = np.asarray(betas, dtype=np.float32)
    W_mlp = np.asarray(W_mlp, dtype=np.float32)

    wqT8 = np.ascontiguousarray(Wq.T * SW).astype(NPF8)
    wkT8 = np.ascontiguousarray(Wk.T * SW).astype(NPF8)
    wq8 = np.ascontiguousarray(Wq * SW).astype(NPF8)
    wk8 = np.ascontiguousarray(Wk * SW).astype(NPF8)
    wmT8 = np.ascontiguousarray(W_mlp.T * SW).astype(NPF8)
    wmT8c = np.empty((D, 2, HID), NPF8)
    wmT8c[:, 0, :] = (W_mlp.T * SW - wmT8.astype(np.float32)).astype(NPF8)
    wmT8c[:, 1, :] = (wmT8.astype(np.float32) / 16.0).astype(NPF8)
    wm8b = np.ascontiguousarray(W_mlp * (SW * C)).astype(NPF8)
    wm8c = np.ascontiguousarray(
        W_mlp * (SW * C) - wm8b.astype(np.float32)).astype(NPF8)
    betap = np.ascontiguousarray(np.broadcast_to(
        (betas / (SW * SW))[None, :], (P, H))).astype(np.float32)
    ident = np.eye(P, dtype=np.float32).astype(NPBF)
    ident8 = np.eye(P, dtype=np.float32).astype(NPF8)

    in_maps = []
    for b in range(B):
        xT = np.ascontiguousarray(x[b].T)
        x8 = xT.astype(NPF8)
        xc = np.empty((D, 2, N), NPF8)
        xc[:, 0, :] = x8
        xc[:, 1, :] = (16.0 * (xT - x8.astype(np.float32))).astype(NPF8)
        in_maps.append({
            "xT8c": xc,
            "wqT8": wqT8, "wkT8": wkT8, "wq8": wq8, "wk8": wk8,
            "wmT8": wmT8, "wmT8c": wmT8c, "wm8b": wm8b, "wm8c": wm8c,
            "betap": betap, "ident": ident, "ident8": ident8,
        })
    return in_maps


def kernel(x, Wq, Wk, betas, W_mlp, _trace=False, _dbg=False):
    key = "nc_dbg" if _dbg else "nc"
    if key not in _CACHE:
        _CACHE[key] = _build(dbg=_dbg)
    nc = _CACHE[key]
    in_maps = _prep(x, Wq, Wk, betas, W_mlp)
    core_ids = list(range(B)) if not _dbg else [0]
    res = run_bass_kernel_spmd(nc, in_maps[:len(core_ids)], core_ids=core_ids,
                               trace=_trace)
    _CACHE["last_result"] = res
    if _dbg:
        return res
    out = np.stack([res.results[b]["out"] for b in range(B)], axis=0)
    return out.astype(np.float32)
